# revision 2
# baseline (speedup 1.0000x reference)
"""Trainium2 Bass kernel v2 — fp16 + PE-accumulated primal, for the CP loop.

Math per image (H=W=1024, 10 iters), tanh/rescaled form (see baseline):
    dual:   qh = relu(qh + gs*t - g1*t_down - g0*t_right)     t pads: -1
    primal: s  = o2 - A - B + A_up + B_left,  A = g1*qh, B = g0*qh, pads 0
            t  = tanh(s/2);  output = s after iteration 10.

Key techniques vs the fp32 baseline:
  * fp16 state: DVE 2-byte tensor_tensor runs at 2x (2.2us/half-image op);
    predicted rel-L2 ~1e-2 vs the 2e-2 gate (numpy bit-level emulation).
  * shifted-PRODUCT reads: A_up/B_left/t_down/t_right are free AP offsets
    into guard planes/cols; only two tiny boundary-row DMAs per iteration.
  * the primal sum runs on the otherwise-idle PE as identity-weight matmuls
    accumulating in PSUM (fp32), in 512-col chunks:
        s_psum = -I@A - I@B + I@o2 + I@A_up + I@B_left
    and the qh sum for the first qh_pe planes likewise:
        qh_psum = I@qh + I@P1 - I@P2 - I@P3
    relu/tanh then read PSUM directly on ScalarE (cheaper access than SBUF).
    This moves most adds off DVE and upgrades the sums to fp32 accumulation.
  * Pool (gpsimd) takes the P2 = g1*t_down product as a plain tensor_tensor
    (the only elementwise form its HW ISA accepts).

Layout: y = 8p + i -> partition p, plane i; x = free col.  Tiles fp16:
    T  [128,9,1025]: t planes 0..7 cols 0..1023, col 1024 = -1 guard,
                     plane 8 = boundary row t[8p+8] (DMA p+1 -> p each iter)
    A  [128,9,1024]: planes 1..8 = g1*qh, plane 0 = boundary A[8p-1]
    B  [128,8,1025]: cols 1..1024 = g0*qh, col 0 = 0 guard
"""

import numpy as np

import concourse.bacc as bacc
import concourse.mybir as mybir
from concourse.tile import TileContext
from concourse import bass_utils

F16 = mybir.dt.float16
F32 = mybir.dt.float32
AF = mybir.ActivationFunctionType
Alu = mybir.AluOpType

B, H, W = 8, 1024, 1024
P = 128
NP = H // P      # 8 planes per partition
WG = W + 1
MAXITER = 10
CH = 512         # matmul moving-operand chunk

_CACHE = {}
LAST_RESULTS = None

DEFAULT_ASSIGN = {
    "s_mode": "pe",   # 'pe' = PSUM-accumulated primal, 'v' = DVE adds
    "b": "v",         # engine for B = g0 * qh ('v', 'g', or 'mix': half each)
    "dn": False,      # True: fold -A-B into Dn on Pool (single +I stationary);
                      # False: 5-term MM with +I/-I stationary swaps
    "qh_pe": 6,       # number of planes (0..8) whose qh-sum runs on PE
    "qh3": False,     # True: R = P1+P2n on DVE -> 3-term qh MM
    "s4": False,      # True: C1 = o2-A on DVE -> 4-term s MM
    "s_dve": 0,       # number of trailing planes whose s-sum runs on DVE
    "v_split": 4,
    "g_split": 4,
    "s_split": 4,
}


def _build(assign=None):
    asg = dict(DEFAULT_ASSIGN)
    if assign:
        asg.update(assign)

    nc = bacc.Bacc("TRN2", target_bir_lowering=False, debug=False)

    o2_d = nc.dram_tensor("o2", [H, W], F16, kind="ExternalInput").ap()
    g0_d = nc.dram_tensor("g0", [H, W], F16, kind="ExternalInput").ap()
    g1_d = nc.dram_tensor("g1", [H, W], F16, kind="ExternalInput").ap()
    eye_d = nc.dram_tensor("eye", [P, P], F16, kind="ExternalInput").ap()
    out_d = nc.dram_tensor("out", [H, W], F16, kind="ExternalOutput").ap()

    o2_v = o2_d.rearrange("(p i) x -> p i x", i=NP)
    g0_v = g0_d.rearrange("(p i) x -> p i x", i=NP)
    g1_v = g1_d.rearrange("(p i) x -> p i x", i=NP)
    out_v = out_d.rearrange("(p i) x -> p i x", i=NP)

    v = nc.vector
    g = nc.gpsimd
    act = nc.scalar
    pe = nc.tensor

    with TileContext(nc) as tc:
        with tc.tile_pool(name="main", bufs=1) as pool, \
             tc.psum_pool(name="ps", bufs=1) as ppool:
            Tt = pool.tile([P, NP + 1, WG], F16)
            QH = pool.tile([P, NP, W], F16)
            O2 = pool.tile([P, NP, W], F16)
            GS = pool.tile([P, NP, W], F16)
            G1 = pool.tile([P, NP, W], F16)
            G0 = pool.tile([P, NP, W], F16)
            At = pool.tile([P, NP + 1, W], F16)
            Bt = pool.tile([P, NP, WG], F16)
            P1t = pool.tile([P, NP, W], F16)
            P2t = pool.tile([P, NP, W], F16)
            P3t = pool.tile([P, NP, W], F16)
            pe_mode = asg["s_mode"] == "pe"
            if pe_mode:
                Eye = pool.tile([P, P], F16)
                pq = [ppool.tile([P, 1, W], F32, name=f"pq{i}") for i in range(2)]
                if asg["qh_pe"]:
                    qpq = [ppool.tile([P, 1, W], F32, name=f"qpq{i}")
                           for i in range(2)]
                NEye = pool.tile([P, P], F16)
                if asg["s4"]:
                    C1 = pool.tile([P, NP, W], F16)

            def spl(n):
                step = NP // n
                return [(i * step, (i + 1) * step) for i in range(n)]

            vs = spl(asg["v_split"])
            gs_ = spl(asg["g_split"])
            ss = spl(asg["s_split"])
            halves = spl(2)

            def t_(lo, hi):
                return Tt[:, lo:hi, 0:W]

            def t_dn(lo, hi):   # t[y+1]; plane 8 = boundary
                return Tt[:, lo + 1 : hi + 1, 0:W]

            def t_rt(lo, hi):   # t[y, x+1]; col W = -1 guard
                return Tt[:, lo:hi, 1 : W + 1]

            def a_(lo, hi):     # A[y] (data planes 1..8)
                return At[:, lo + 1 : hi + 1, :]

            def a_up(lo, hi):   # A[y-1] (plane 0 = boundary)
                return At[:, lo:hi, :]

            def b_(lo, hi):     # B[y] (data cols 1..W)
                return Bt[:, lo:hi, 1 : W + 1]

            def b_lf(lo, hi):   # B[y, x-1] (col 0 = 0 guard)
                return Bt[:, lo:hi, 0:W]

            # --- setup ---
            v.memset(Tt[:, :, :], -1.0)
            v.memset(QH[:, :, :], 0.0)
            v.memset(At[:, 0, :], 0.0)
            v.memset(Bt[:, :, 0:1], 0.0)
            nc.sync.dma_start(out=O2[:, :, :], in_=o2_v)
            nc.sync.dma_start(out=G0[:, :, :], in_=g0_v)
            nc.sync.dma_start(out=G1[:, :, :], in_=g1_v)
            if pe_mode:
                nc.sync.dma_start(out=Eye[:, :], in_=eye_d)
                v.tensor_scalar_mul(NEye[:, :], Eye[:, :], -1.0)
            v.tensor_add(GS[:, :, :], G0[:, :, :], G1[:, :, :])
            for lo, hi in halves:
                act.activation(t_(lo, hi), O2[:, lo:hi, :], AF.Tanh, scale=0.5)
            nc.sync.dma_start(out=Tt[0 : P - 1, NP, 0:W], in_=Tt[1:P, 0, 0:W])

            for it in range(MAXITER):
                last = it == MAXITER - 1

                # ---- dual: qh' = relu(qh + P1 - P2 - P3) ----
                # Products are positive (Pool only supports plain
                # tensor_tensor on HW); minus signs come from DVE subtracts or
                # the -I stationary on PE planes.
                qpe = asg["qh_pe"] if pe_mode else 0
                for lo, hi in gs_:
                    g.tensor_mul(P2t[:, lo:hi, :], t_dn(lo, hi), G1[:, lo:hi, :])
                for lo, hi in vs:
                    v.tensor_mul(P1t[:, lo:hi, :], GS[:, lo:hi, :], t_(lo, hi))
                for lo, hi in vs:
                    v.tensor_mul(P3t[:, lo:hi, :], G0[:, lo:hi, :], t_rt(lo, hi))
                for j in range(qpe):            # PE planes: qh-sum in PSUM
                    qp = qpq[j % 2]
                    for c in range(0, W, CH):
                        pslab = qp[:, 0, c : c + CH]
                        terms = [] if it == 0 else [(Eye, QH[:, j, c : c + CH])]
                        terms += [(Eye, P1t[:, j, c : c + CH]),
                                  (NEye, P2t[:, j, c : c + CH]),
                                  (NEye, P3t[:, j, c : c + CH])]
                        for k, (w, src) in enumerate(terms):
                            pe.matmul(pslab, w[:, :], src, start=(k == 0),
                                      stop=(k == len(terms) - 1))
                    act.activation(QH[:, j : j + 1, :], qp[:, :, :], AF.Relu)
                if qpe < NP:                    # DVE planes: in-place adds
                    dvs = [(max(lo, qpe), hi) for lo, hi in vs if hi > qpe]
                    if it == 0:
                        for lo, hi in dvs:
                            v.tensor_sub(QH[:, lo:hi, :], P1t[:, lo:hi, :],
                                         P3t[:, lo:hi, :])
                    else:
                        for lo, hi in dvs:
                            v.tensor_add(QH[:, lo:hi, :], QH[:, lo:hi, :],
                                         P1t[:, lo:hi, :])
                        for lo, hi in dvs:
                            v.tensor_sub(QH[:, lo:hi, :], QH[:, lo:hi, :],
                                         P3t[:, lo:hi, :])
                    for lo, hi in dvs:
                        v.tensor_sub(QH[:, lo:hi, :], QH[:, lo:hi, :],
                                     P2t[:, lo:hi, :])
                    for lo, hi in [(max(lo, qpe), hi) for lo, hi in ss
                                   if hi > qpe]:
                        act.activation(QH[:, lo:hi, :], QH[:, lo:hi, :], AF.Relu)

                # ---- primal ----
                for lo, hi in vs:
                    v.tensor_mul(a_(lo, hi), G1[:, lo:hi, :], QH[:, lo:hi, :])
                nc.sync.dma_start(out=At[1:P, 0, :], in_=At[0 : P - 1, NP, :])
                bsplits = vs if asg["b"] == "v" else gs_
                for k, (lo, hi) in enumerate(bsplits):
                    on_v = asg["b"] == "v" or (asg["b"] == "mix"
                                               and k < len(bsplits) // 2)
                    if on_v:
                        v.tensor_mul(b_(lo, hi), G0[:, lo:hi, :], QH[:, lo:hi, :])
                    else:
                        g.tensor_mul(b_(lo, hi), G0[:, lo:hi, :], QH[:, lo:hi, :])

                if pe_mode:
                    # s accumulates on PE in PSUM per 512-col chunk:
                    # dn=True:  s = I@o2 + I@Dn + I@A_up + I@B_left,
                    #           with Dn = -A-B from one Pool STT
                    # dn=False: s = -I@A - I@B + I@o2 + I@A_up + I@B_left
                    assert not asg["dn"], "dn mode needs Pool STT (not on HW)"
                    if asg["s4"]:
                        for lo, hi in vs:
                            v.tensor_sub(C1[:, lo:hi, :], O2[:, lo:hi, :],
                                         a_(lo, hi))
                    sdve = asg["s_dve"]
                    if sdve:                    # trailing planes on DVE
                        lo, hi = NP - sdve, NP
                        v.tensor_sub(t_(lo, hi), O2[:, lo:hi, :], a_(lo, hi))
                        v.tensor_sub(t_(lo, hi), t_(lo, hi), b_(lo, hi))
                        v.tensor_add(t_(lo, hi), t_(lo, hi), a_up(lo, hi))
                        v.tensor_add(t_(lo, hi), t_(lo, hi), b_lf(lo, hi))
                        if not last:
                            act.activation(t_(lo, hi), t_(lo, hi), AF.Tanh,
                                           scale=0.5)
                    for j in range(NP - sdve):  # per plane: 2 banks ping-pong
                        ps = pq[j % 2]
                        for c in range(0, W, CH):
                            pslab = ps[:, 0, c : c + CH]
                            if asg["dn"]:
                                terms = [(Eye, O2[:, j, c : c + CH]),
                                         (Eye, Dn[:, j, c : c + CH]),
                                         (Eye, At[:, j, c : c + CH]),
                                         (Eye, Bt[:, j, c : c + CH])]
                            elif asg["s4"]:
                                terms = [(Eye, C1[:, j, c : c + CH]),
                                         (NEye, b_(j, j + 1)[:, 0, c : c + CH]),
                                         (Eye, At[:, j, c : c + CH]),
                                         (Eye, Bt[:, j, c : c + CH])]
                            else:
                                terms = [(NEye, a_(j, j + 1)[:, 0, c : c + CH]),
                                         (NEye, b_(j, j + 1)[:, 0, c : c + CH]),
                                         (Eye, O2[:, j, c : c + CH]),
                                         (Eye, At[:, j, c : c + CH]),
                                         (Eye, Bt[:, j, c : c + CH])]
                            for k, (w, src) in enumerate(terms):
                                pe.matmul(pslab, w[:, :], src,
                                          start=(k == 0),
                                          stop=(k == len(terms) - 1))
                        fn = AF.Copy if last else AF.Tanh
                        kw = {} if last else {"scale": 0.5}
                        act.activation(Tt[:, j : j + 1, 0:W], ps[:, :, :],
                                       fn, **kw)
                        if j == 0 and not last:
                            nc.sync.dma_start(
                                out=Tt[0 : P - 1, NP, 0:W], in_=Tt[1:P, 0, 0:W]
                            )
                else:
                    for lo, hi in reversed(vs):
                        v.tensor_sub(t_(lo, hi), O2[:, lo:hi, :], a_(lo, hi))
                    for lo, hi in reversed(vs):
                        v.tensor_sub(t_(lo, hi), t_(lo, hi), b_(lo, hi))
                    for lo, hi in reversed(vs):
                        v.tensor_add(t_(lo, hi), t_(lo, hi), a_up(lo, hi))
                    for lo, hi in reversed(vs):
                        v.tensor_add(t_(lo, hi), t_(lo, hi), b_lf(lo, hi))
                    if not last:
                        for lo, hi in ss:
                            act.activation(t_(lo, hi), t_(lo, hi), AF.Tanh,
                                           scale=0.5)
                            if lo == 0:
                                nc.sync.dma_start(
                                    out=Tt[0 : P - 1, NP, 0:W], in_=Tt[1:P, 0, 0:W]
                                )

            nc.sync.dma_start(out=out_v, in_=Tt[:, 0:NP, 0:W])

    nc.compile()
    return nc


def kernel(o, vector_field, nabla_w, div_w):
    global LAST_RESULTS
    if "nc" not in _CACHE:
        _CACHE["nc"] = _build()
    nc = _CACHE["nc"]

    o2 = np.ascontiguousarray(
        (2.0 * np.asarray(o, dtype=np.float32)[:, 0]).astype(np.float16)
    )
    vf = np.asarray(vector_field, dtype=np.float32)
    s = np.float32(1.0 / np.sqrt(2.0))
    g0 = np.ascontiguousarray((vf[:, :, 0] * s).astype(np.float16))
    g1 = np.ascontiguousarray((vf[:, :, 1] * s).astype(np.float16))
    eye = np.eye(P, dtype=np.float16)

    in_maps = [{"o2": o2[b], "g0": g0, "g1": g1, "eye": eye} for b in range(B)]
    res = bass_utils.run_bass_kernel_spmd(nc, in_maps, core_ids=list(range(B)))
    LAST_RESULTS = res
    return np.stack([r["out"] for r in res.results]).astype(np.float32)


# revision 5
# speedup vs baseline: 1.1164x; 1.1164x over previous
"""Trainium2 Bass kernel v2 — fp16 + PE-accumulated primal, for the CP loop.

Math per image (H=W=1024, 10 iters), tanh/rescaled form (see baseline):
    dual:   qh = relu(qh + gs*t - g1*t_down - g0*t_right)     t pads: -1
    primal: s  = o2 - A - B + A_up + B_left,  A = g1*qh, B = g0*qh, pads 0
            t  = tanh(s/2);  output = s after iteration 10.

Key techniques vs the fp32 baseline:
  * fp16 state: DVE 2-byte tensor_tensor runs at 2x (2.2us/half-image op);
    predicted rel-L2 ~1e-2 vs the 2e-2 gate (numpy bit-level emulation).
  * shifted-PRODUCT reads: A_up/B_left/t_down/t_right are free AP offsets
    into guard planes/cols; only two tiny boundary-row DMAs per iteration.
  * the primal sum runs on the otherwise-idle PE as identity-weight matmuls
    accumulating in PSUM (fp32), in 512-col chunks:
        s_psum = -I@A - I@B + I@o2 + I@A_up + I@B_left
    and the qh sum for the first qh_pe planes likewise:
        qh_psum = I@qh + I@P1 - I@P2 - I@P3
    relu/tanh then read PSUM directly on ScalarE (cheaper access than SBUF).
    This moves most adds off DVE and upgrades the sums to fp32 accumulation.
  * Pool (gpsimd) takes the P2 = g1*t_down product as a plain tensor_tensor
    (the only elementwise form its HW ISA accepts).

Layout: y = 8p + i -> partition p, plane i; x = free col.  Tiles fp16:
    T  [128,9,1025]: t planes 0..7 cols 0..1023, col 1024 = -1 guard,
                     plane 8 = boundary row t[8p+8] (DMA p+1 -> p each iter)
    A  [128,9,1024]: planes 1..8 = g1*qh, plane 0 = boundary A[8p-1]
    B  [128,8,1025]: cols 1..1024 = g0*qh, col 0 = 0 guard
"""

import numpy as np

import concourse.bacc as bacc
import concourse.mybir as mybir
from concourse.tile import TileContext
from concourse import bass_utils

F16 = mybir.dt.float16
F32 = mybir.dt.float32
AF = mybir.ActivationFunctionType
Alu = mybir.AluOpType

B, H, W = 8, 1024, 1024
P = 128
NP = H // P      # 8 planes per partition
WG = W + 1
MAXITER = 10
CH = 512         # matmul moving-operand chunk

_CACHE = {}
LAST_RESULTS = None

DEFAULT_ASSIGN = {
    "s_mode": "pe",   # 'pe' = PSUM-accumulated primal, 'v' = DVE adds
    "b": "v",         # engine for B = g0 * qh ('v', 'g', or 'mix': half each)
    "dn": False,      # True: fold -A-B into Dn on Pool (single +I stationary);
                      # False: 5-term MM with +I/-I stationary swaps
    "qh_pe": 6,       # number of planes (0..8) whose qh-sum runs on PE
    "qh3": False,     # True: R = P1+P2n on DVE -> 3-term qh MM
    "s4": False,      # True: C1 = o2-A on DVE -> 4-term s MM
    "s_dve": 1,       # number of trailing planes whose s-sum runs on DVE
    "p2": "mix",      # engine for P2 = g1*t_down ('g', 'v', 'mix')
    # s-plane order: run plane 0 third so its A_up boundary-row DMA (which
    # needs the last A quarter) is complete by the time its matmuls issue
    "s_order": [1, 2, 0, 3, 4, 5, 6, 7],
    "v_split": 4,
    "g_split": 4,
    "s_split": 4,
}


def _build(assign=None):
    asg = dict(DEFAULT_ASSIGN)
    if assign:
        asg.update(assign)

    nc = bacc.Bacc("TRN2", target_bir_lowering=False, debug=False)

    o2_d = nc.dram_tensor("o2", [H, W], F16, kind="ExternalInput").ap()
    g0_d = nc.dram_tensor("g0", [H, W], F16, kind="ExternalInput").ap()
    g1_d = nc.dram_tensor("g1", [H, W], F16, kind="ExternalInput").ap()
    eye_d = nc.dram_tensor("eye", [P, P], F16, kind="ExternalInput").ap()
    out_d = nc.dram_tensor("out", [H, W], F16, kind="ExternalOutput").ap()

    o2_v = o2_d.rearrange("(p i) x -> p i x", i=NP)
    g0_v = g0_d.rearrange("(p i) x -> p i x", i=NP)
    g1_v = g1_d.rearrange("(p i) x -> p i x", i=NP)
    out_v = out_d.rearrange("(p i) x -> p i x", i=NP)

    v = nc.vector
    g = nc.gpsimd
    act = nc.scalar
    pe = nc.tensor

    with TileContext(nc) as tc:
        with tc.tile_pool(name="main", bufs=1) as pool, \
             tc.psum_pool(name="ps", bufs=1) as ppool:
            Tt = pool.tile([P, NP + 1, WG], F16)
            QH = pool.tile([P, NP, W], F16)
            O2 = pool.tile([P, NP, W], F16)
            GS = pool.tile([P, NP, W], F16)
            G1 = pool.tile([P, NP, W], F16)
            G0 = pool.tile([P, NP, W], F16)
            At = pool.tile([P, NP + 1, W], F16)
            Bt = pool.tile([P, NP, WG], F16)
            P1t = pool.tile([P, NP, W], F16)
            P2t = pool.tile([P, NP, W], F16)
            P3t = pool.tile([P, NP, W], F16)
            pe_mode = asg["s_mode"] == "pe"
            if pe_mode:
                Eye = pool.tile([P, P], F16)
                pq = [ppool.tile([P, 1, W], F32, name=f"pq{i}") for i in range(2)]
                if asg["qh_pe"]:
                    qpq = [ppool.tile([P, 1, W], F32, name=f"qpq{i}")
                           for i in range(2)]
                NEye = pool.tile([P, P], F16)
                if asg["s4"]:
                    C1 = pool.tile([P, NP, W], F16)

            def spl(n):
                step = NP // n
                return [(i * step, (i + 1) * step) for i in range(n)]

            vs = spl(asg["v_split"])
            gs_ = spl(asg["g_split"])
            ss = spl(asg["s_split"])
            halves = spl(2)

            def t_(lo, hi):
                return Tt[:, lo:hi, 0:W]

            def t_dn(lo, hi):   # t[y+1]; plane 8 = boundary
                return Tt[:, lo + 1 : hi + 1, 0:W]

            def t_rt(lo, hi):   # t[y, x+1]; col W = -1 guard
                return Tt[:, lo:hi, 1 : W + 1]

            def a_(lo, hi):     # A[y] (data planes 1..8)
                return At[:, lo + 1 : hi + 1, :]

            def a_up(lo, hi):   # A[y-1] (plane 0 = boundary)
                return At[:, lo:hi, :]

            def b_(lo, hi):     # B[y] (data cols 1..W)
                return Bt[:, lo:hi, 1 : W + 1]

            def b_lf(lo, hi):   # B[y, x-1] (col 0 = 0 guard)
                return Bt[:, lo:hi, 0:W]

            # --- setup ---
            v.memset(Tt[:, :, :], -1.0)
            v.memset(QH[:, :, :], 0.0)
            v.memset(At[:, 0, :], 0.0)
            v.memset(Bt[:, :, 0:1], 0.0)
            nc.sync.dma_start(out=O2[:, :, :], in_=o2_v)
            nc.sync.dma_start(out=G0[:, :, :], in_=g0_v)
            nc.sync.dma_start(out=G1[:, :, :], in_=g1_v)
            if pe_mode:
                nc.sync.dma_start(out=Eye[:, :], in_=eye_d)
                v.tensor_scalar_mul(NEye[:, :], Eye[:, :], -1.0)
            v.tensor_add(GS[:, :, :], G0[:, :, :], G1[:, :, :])
            for lo, hi in halves:
                act.activation(t_(lo, hi), O2[:, lo:hi, :], AF.Tanh, scale=0.5)
            nc.sync.dma_start(out=Tt[0 : P - 1, NP, 0:W], in_=Tt[1:P, 0, 0:W])

            for it in range(MAXITER):
                last = it == MAXITER - 1

                # ---- dual: qh' = relu(qh + P1 - P2 - P3) ----
                # Products are positive (Pool only supports plain
                # tensor_tensor on HW); minus signs come from DVE subtracts or
                # the -I stationary on PE planes.
                qpe = asg["qh_pe"] if pe_mode else 0
                p2splits = gs_ if asg["p2"] == "g" else vs
                for k, (lo, hi) in enumerate(p2splits):
                    on_g = asg["p2"] == "g" or (asg["p2"] == "mix"
                                                and k >= len(p2splits) // 2)
                    if on_g:
                        g.tensor_mul(P2t[:, lo:hi, :], t_dn(lo, hi),
                                     G1[:, lo:hi, :])
                    else:
                        v.tensor_mul(P2t[:, lo:hi, :], t_dn(lo, hi),
                                     G1[:, lo:hi, :])
                for lo, hi in vs:
                    v.tensor_mul(P1t[:, lo:hi, :], GS[:, lo:hi, :], t_(lo, hi))
                for lo, hi in vs:
                    v.tensor_mul(P3t[:, lo:hi, :], G0[:, lo:hi, :], t_rt(lo, hi))
                for j in range(qpe):            # PE planes: qh-sum in PSUM
                    qp = qpq[j % 2]
                    for c in range(0, W, CH):
                        pslab = qp[:, 0, c : c + CH]
                        terms = [] if it == 0 else [(Eye, QH[:, j, c : c + CH])]
                        terms += [(Eye, P1t[:, j, c : c + CH]),
                                  (NEye, P2t[:, j, c : c + CH]),
                                  (NEye, P3t[:, j, c : c + CH])]
                        for k, (w, src) in enumerate(terms):
                            pe.matmul(pslab, w[:, :], src, start=(k == 0),
                                      stop=(k == len(terms) - 1))
                    act.activation(QH[:, j : j + 1, :], qp[:, :, :], AF.Relu)
                if qpe < NP:                    # DVE planes: in-place adds
                    dvs = [(max(lo, qpe), hi) for lo, hi in vs if hi > qpe]
                    if it == 0:
                        for lo, hi in dvs:
                            v.tensor_sub(QH[:, lo:hi, :], P1t[:, lo:hi, :],
                                         P3t[:, lo:hi, :])
                    else:
                        for lo, hi in dvs:
                            v.tensor_add(QH[:, lo:hi, :], QH[:, lo:hi, :],
                                         P1t[:, lo:hi, :])
                        for lo, hi in dvs:
                            v.tensor_sub(QH[:, lo:hi, :], QH[:, lo:hi, :],
                                         P3t[:, lo:hi, :])
                    for lo, hi in dvs:
                        v.tensor_sub(QH[:, lo:hi, :], QH[:, lo:hi, :],
                                     P2t[:, lo:hi, :])
                    for lo, hi in [(max(lo, qpe), hi) for lo, hi in ss
                                   if hi > qpe]:
                        act.activation(QH[:, lo:hi, :], QH[:, lo:hi, :], AF.Relu)

                # ---- primal ----
                for lo, hi in vs:
                    v.tensor_mul(a_(lo, hi), G1[:, lo:hi, :], QH[:, lo:hi, :])
                nc.sync.dma_start(out=At[1:P, 0, :], in_=At[0 : P - 1, NP, :])
                bsplits = vs if asg["b"] == "v" else gs_
                for k, (lo, hi) in enumerate(bsplits):
                    on_v = asg["b"] == "v" or (asg["b"] == "mix"
                                               and k < len(bsplits) // 2)
                    if on_v:
                        v.tensor_mul(b_(lo, hi), G0[:, lo:hi, :], QH[:, lo:hi, :])
                    else:
                        g.tensor_mul(b_(lo, hi), G0[:, lo:hi, :], QH[:, lo:hi, :])

                if pe_mode:
                    # s accumulates on PE in PSUM per 512-col chunk:
                    # dn=True:  s = I@o2 + I@Dn + I@A_up + I@B_left,
                    #           with Dn = -A-B from one Pool STT
                    # dn=False: s = -I@A - I@B + I@o2 + I@A_up + I@B_left
                    assert not asg["dn"], "dn mode needs Pool STT (not on HW)"
                    if asg["s4"]:
                        for lo, hi in vs:
                            v.tensor_sub(C1[:, lo:hi, :], O2[:, lo:hi, :],
                                         a_(lo, hi))
                    sdve = asg["s_dve"]
                    if sdve:                    # trailing planes on DVE
                        lo, hi = NP - sdve, NP
                        v.tensor_sub(t_(lo, hi), O2[:, lo:hi, :], a_(lo, hi))
                        v.tensor_sub(t_(lo, hi), t_(lo, hi), b_(lo, hi))
                        v.tensor_add(t_(lo, hi), t_(lo, hi), a_up(lo, hi))
                        v.tensor_add(t_(lo, hi), t_(lo, hi), b_lf(lo, hi))
                        if not last:
                            act.activation(t_(lo, hi), t_(lo, hi), AF.Tanh,
                                           scale=0.5)
                    n_s = NP - sdve
                    so = asg["s_order"]
                    if isinstance(so, list):
                        order = [j for j in so if j < n_s]
                    else:
                        order = {0: list(range(n_s)),
                                 1: [1, 2, 3, 0] + list(range(4, n_s)),
                                 2: list(reversed(range(n_s)))}[so]
                    for j in order:             # plane: 2-bank ping-pong
                        ps = pq[j % 2]
                        for c in range(0, W, CH):
                            pslab = ps[:, 0, c : c + CH]
                            if asg["dn"]:
                                terms = [(Eye, O2[:, j, c : c + CH]),
                                         (Eye, Dn[:, j, c : c + CH]),
                                         (Eye, At[:, j, c : c + CH]),
                                         (Eye, Bt[:, j, c : c + CH])]
                            elif asg["s4"]:
                                terms = [(Eye, C1[:, j, c : c + CH]),
                                         (NEye, b_(j, j + 1)[:, 0, c : c + CH]),
                                         (Eye, At[:, j, c : c + CH]),
                                         (Eye, Bt[:, j, c : c + CH])]
                            else:
                                terms = [(NEye, a_(j, j + 1)[:, 0, c : c + CH]),
                                         (NEye, b_(j, j + 1)[:, 0, c : c + CH]),
                                         (Eye, O2[:, j, c : c + CH]),
                                         (Eye, At[:, j, c : c + CH]),
                                         (Eye, Bt[:, j, c : c + CH])]
                            for k, (w, src) in enumerate(terms):
                                pe.matmul(pslab, w[:, :], src,
                                          start=(k == 0),
                                          stop=(k == len(terms) - 1))
                        fn = AF.Copy if last else AF.Tanh
                        kw = {} if last else {"scale": 0.5}
                        act.activation(Tt[:, j : j + 1, 0:W], ps[:, :, :],
                                       fn, **kw)
                        if j == 0 and not last:
                            nc.sync.dma_start(
                                out=Tt[0 : P - 1, NP, 0:W], in_=Tt[1:P, 0, 0:W]
                            )
                else:
                    for lo, hi in reversed(vs):
                        v.tensor_sub(t_(lo, hi), O2[:, lo:hi, :], a_(lo, hi))
                    for lo, hi in reversed(vs):
                        v.tensor_sub(t_(lo, hi), t_(lo, hi), b_(lo, hi))
                    for lo, hi in reversed(vs):
                        v.tensor_add(t_(lo, hi), t_(lo, hi), a_up(lo, hi))
                    for lo, hi in reversed(vs):
                        v.tensor_add(t_(lo, hi), t_(lo, hi), b_lf(lo, hi))
                    if not last:
                        for lo, hi in ss:
                            act.activation(t_(lo, hi), t_(lo, hi), AF.Tanh,
                                           scale=0.5)
                            if lo == 0:
                                nc.sync.dma_start(
                                    out=Tt[0 : P - 1, NP, 0:W], in_=Tt[1:P, 0, 0:W]
                                )

            nc.sync.dma_start(out=out_v, in_=Tt[:, 0:NP, 0:W])

    nc.compile()
    return nc


def kernel(o, vector_field, nabla_w, div_w):
    global LAST_RESULTS
    if "nc" not in _CACHE:
        _CACHE["nc"] = _build()
    nc = _CACHE["nc"]

    o2 = np.ascontiguousarray(
        (2.0 * np.asarray(o, dtype=np.float32)[:, 0]).astype(np.float16)
    )
    vf = np.asarray(vector_field, dtype=np.float32)
    s = np.float32(1.0 / np.sqrt(2.0))
    g0 = np.ascontiguousarray((vf[:, :, 0] * s).astype(np.float16))
    g1 = np.ascontiguousarray((vf[:, :, 1] * s).astype(np.float16))
    eye = np.eye(P, dtype=np.float16)

    in_maps = [{"o2": o2[b], "g0": g0, "g1": g1, "eye": eye} for b in range(B)]
    res = bass_utils.run_bass_kernel_spmd(nc, in_maps, core_ids=list(range(B)))
    LAST_RESULTS = res
    return np.stack([r["out"] for r in res.results]).astype(np.float32)


# revision 7
# speedup vs baseline: 1.1304x; 1.0125x over previous
"""Trainium2 Bass kernel v2 — fp16 + PE-accumulated primal, for the CP loop.

Math per image (H=W=1024, 10 iters), tanh/rescaled form (see baseline):
    dual:   qh = relu(qh + gs*t - g1*t_down - g0*t_right)     t pads: -1
    primal: s  = o2 - A - B + A_up + B_left,  A = g1*qh, B = g0*qh, pads 0
            t  = tanh(s/2);  output = s after iteration 10.

Measured: 310,054 ns timeline-sim (fp32 baseline: 1,122,789 ns, 3.6x), HW
rel-L2 9.18e-3 vs the 2e-2 gate (the error is relu-boundary chaos amplifying
fp16 rounding -- an ensemble statistic, stable across reruns).

Key techniques vs the fp32 baseline:
  * fp16 state: DVE 2-byte tensor_tensor runs at 2x (2.2us/half-image op);
    numpy bit-level emulation predicted rel-L2 ~1e-2, HW measures 9.2e-3.
  * shifted-PRODUCT reads: A_up/B_left/t_down/t_right are free AP offsets
    into guard planes/cols; only two tiny boundary-row DMAs per iteration.
  * the primal sum runs on the otherwise-idle PE as identity-weight matmuls
    accumulating in PSUM (fp32), in 512-col chunks:
        s_psum = -I@A - I@B + I@o2 + I@A_up + I@B_left
    and the qh sum for the first qh_pe planes likewise:
        qh_psum = I@qh + I@P1 - I@P2 - I@P3
    relu/tanh then read PSUM directly on ScalarE (cheaper access than SBUF).
    This moves most adds off DVE and upgrades the sums to fp32 accumulation.
  * Pool (gpsimd) takes the P2 = g1*t_down product as a plain tensor_tensor
    (the only elementwise form its HW ISA accepts).

Layout: y = 8p + i -> partition p, plane i; x = free col.  Tiles fp16:
    T  [128,9,1025]: t planes 0..7 cols 0..1023, col 1024 = -1 guard,
                     plane 8 = boundary row t[8p+8] (DMA p+1 -> p each iter)
    A  [128,9,1024]: planes 1..8 = g1*qh, plane 0 = boundary A[8p-1]
    B  [128,8,1025]: cols 1..1024 = g0*qh, col 0 = 0 guard
"""

import numpy as np

import concourse.bacc as bacc
import concourse.mybir as mybir
from concourse.tile import TileContext
from concourse import bass_utils

F16 = mybir.dt.float16
F32 = mybir.dt.float32
AF = mybir.ActivationFunctionType
Alu = mybir.AluOpType

B, H, W = 8, 1024, 1024
P = 128
NP = H // P      # 8 planes per partition
WG = W + 1
MAXITER = 10
CH = 512         # matmul moving-operand chunk

_CACHE = {}
LAST_RESULTS = None

DEFAULT_ASSIGN = {
    "s_mode": "pe",   # 'pe' = PSUM-accumulated primal, 'v' = DVE adds
    "b": "mix",       # engine for B = g0 * qh ('v', 'g', or 'mix': half each)
    "dn": False,      # True: fold -A-B into Dn on Pool (single +I stationary);
                      # False: 5-term MM with +I/-I stationary swaps
    "qh_pe": 6,       # number of planes (0..8) whose qh-sum runs on PE
    "qh3": False,     # True: R = P1+P2n on DVE -> 3-term qh MM
    "s4": False,      # True: C1 = o2-A on DVE -> 4-term s MM
    "s_dve": 1,       # number of trailing planes whose s-sum runs on DVE
    "p2": "mix",      # engine for P2 = g1*t_down ('g', 'v', 'mix')
    # s-plane order: run plane 0 third so its A_up boundary-row DMA (which
    # needs the last A quarter) is complete by the time its matmuls issue
    "s_order": [1, 2, 0, 3, 4, 5, 6, 7],
    "v_split": 4,
    "g_split": 4,
    "s_split": 4,
}


def _build(assign=None):
    asg = dict(DEFAULT_ASSIGN)
    if assign:
        asg.update(assign)

    nc = bacc.Bacc("TRN2", target_bir_lowering=False, debug=False)

    o2_d = nc.dram_tensor("o2", [H, W], F16, kind="ExternalInput").ap()
    g0_d = nc.dram_tensor("g0", [H, W], F16, kind="ExternalInput").ap()
    g1_d = nc.dram_tensor("g1", [H, W], F16, kind="ExternalInput").ap()
    eye_d = nc.dram_tensor("eye", [P, P], F16, kind="ExternalInput").ap()
    out_d = nc.dram_tensor("out", [H, W], F16, kind="ExternalOutput").ap()

    o2_v = o2_d.rearrange("(p i) x -> p i x", i=NP)
    g0_v = g0_d.rearrange("(p i) x -> p i x", i=NP)
    g1_v = g1_d.rearrange("(p i) x -> p i x", i=NP)
    out_v = out_d.rearrange("(p i) x -> p i x", i=NP)

    v = nc.vector
    g = nc.gpsimd
    act = nc.scalar
    pe = nc.tensor

    with TileContext(nc) as tc:
        with tc.tile_pool(name="main", bufs=1) as pool, \
             tc.psum_pool(name="ps", bufs=1) as ppool:
            Tt = pool.tile([P, NP + 1, WG], F16)
            QH = pool.tile([P, NP, W], F16)
            O2 = pool.tile([P, NP, W], F16)
            GS = pool.tile([P, NP, W], F16)
            G1 = pool.tile([P, NP, W], F16)
            G0 = pool.tile([P, NP, W], F16)
            At = pool.tile([P, NP + 1, W], F16)
            Bt = pool.tile([P, NP, WG], F16)
            P1t = pool.tile([P, NP, W], F16)
            P2t = pool.tile([P, NP, W], F16)
            P3t = pool.tile([P, NP, W], F16)
            pe_mode = asg["s_mode"] == "pe"
            if pe_mode:
                Eye = pool.tile([P, P], F16)
                pq = [ppool.tile([P, 1, W], F32, name=f"pq{i}") for i in range(2)]
                if asg["qh_pe"]:
                    qpq = [ppool.tile([P, 1, W], F32, name=f"qpq{i}")
                           for i in range(2)]
                NEye = pool.tile([P, P], F16)
                if asg["s4"]:
                    C1 = pool.tile([P, NP, W], F16)

            def spl(n):
                step = NP // n
                return [(i * step, (i + 1) * step) for i in range(n)]

            vs = spl(asg["v_split"])
            gs_ = spl(asg["g_split"])
            ss = spl(asg["s_split"])
            halves = spl(2)

            def t_(lo, hi):
                return Tt[:, lo:hi, 0:W]

            def t_dn(lo, hi):   # t[y+1]; plane 8 = boundary
                return Tt[:, lo + 1 : hi + 1, 0:W]

            def t_rt(lo, hi):   # t[y, x+1]; col W = -1 guard
                return Tt[:, lo:hi, 1 : W + 1]

            def a_(lo, hi):     # A[y] (data planes 1..8)
                return At[:, lo + 1 : hi + 1, :]

            def a_up(lo, hi):   # A[y-1] (plane 0 = boundary)
                return At[:, lo:hi, :]

            def b_(lo, hi):     # B[y] (data cols 1..W)
                return Bt[:, lo:hi, 1 : W + 1]

            def b_lf(lo, hi):   # B[y, x-1] (col 0 = 0 guard)
                return Bt[:, lo:hi, 0:W]

            # --- setup ---
            v.memset(Tt[:, :, :], -1.0)
            v.memset(QH[:, :, :], 0.0)
            v.memset(At[:, 0, :], 0.0)
            v.memset(Bt[:, :, 0:1], 0.0)
            nc.sync.dma_start(out=O2[:, :, :], in_=o2_v)
            nc.sync.dma_start(out=G0[:, :, :], in_=g0_v)
            nc.sync.dma_start(out=G1[:, :, :], in_=g1_v)
            if pe_mode:
                nc.sync.dma_start(out=Eye[:, :], in_=eye_d)
                v.tensor_scalar_mul(NEye[:, :], Eye[:, :], -1.0)
            v.tensor_add(GS[:, :, :], G0[:, :, :], G1[:, :, :])
            for lo, hi in halves:
                act.activation(t_(lo, hi), O2[:, lo:hi, :], AF.Tanh, scale=0.5)
            nc.sync.dma_start(out=Tt[0 : P - 1, NP, 0:W], in_=Tt[1:P, 0, 0:W])

            for it in range(MAXITER):
                last = it == MAXITER - 1

                # ---- dual: qh' = relu(qh + P1 - P2 - P3) ----
                # Products are positive (Pool only supports plain
                # tensor_tensor on HW); minus signs come from DVE subtracts or
                # the -I stationary on PE planes.
                qpe = asg["qh_pe"] if pe_mode else 0
                p2splits = gs_ if asg["p2"] == "g" else vs
                for k, (lo, hi) in enumerate(p2splits):
                    on_g = asg["p2"] == "g" or (asg["p2"] == "mix"
                                                and k >= len(p2splits) // 2)
                    if on_g:
                        g.tensor_mul(P2t[:, lo:hi, :], t_dn(lo, hi),
                                     G1[:, lo:hi, :])
                    else:
                        v.tensor_mul(P2t[:, lo:hi, :], t_dn(lo, hi),
                                     G1[:, lo:hi, :])
                for lo, hi in vs:
                    v.tensor_mul(P1t[:, lo:hi, :], GS[:, lo:hi, :], t_(lo, hi))
                for lo, hi in vs:
                    v.tensor_mul(P3t[:, lo:hi, :], G0[:, lo:hi, :], t_rt(lo, hi))
                for j in range(qpe):            # PE planes: qh-sum in PSUM
                    qp = qpq[j % 2]
                    for c in range(0, W, CH):
                        pslab = qp[:, 0, c : c + CH]
                        terms = [] if it == 0 else [(Eye, QH[:, j, c : c + CH])]
                        terms += [(Eye, P1t[:, j, c : c + CH]),
                                  (NEye, P2t[:, j, c : c + CH]),
                                  (NEye, P3t[:, j, c : c + CH])]
                        for k, (w, src) in enumerate(terms):
                            pe.matmul(pslab, w[:, :], src, start=(k == 0),
                                      stop=(k == len(terms) - 1))
                    act.activation(QH[:, j : j + 1, :], qp[:, :, :], AF.Relu)
                if qpe < NP:                    # DVE planes: in-place adds
                    dvs = [(max(lo, qpe), hi) for lo, hi in vs if hi > qpe]
                    if it == 0:
                        for lo, hi in dvs:
                            v.tensor_sub(QH[:, lo:hi, :], P1t[:, lo:hi, :],
                                         P3t[:, lo:hi, :])
                    else:
                        for lo, hi in dvs:
                            v.tensor_add(QH[:, lo:hi, :], QH[:, lo:hi, :],
                                         P1t[:, lo:hi, :])
                        for lo, hi in dvs:
                            v.tensor_sub(QH[:, lo:hi, :], QH[:, lo:hi, :],
                                         P3t[:, lo:hi, :])
                    for lo, hi in dvs:
                        v.tensor_sub(QH[:, lo:hi, :], QH[:, lo:hi, :],
                                     P2t[:, lo:hi, :])
                    for lo, hi in [(max(lo, qpe), hi) for lo, hi in ss
                                   if hi > qpe]:
                        act.activation(QH[:, lo:hi, :], QH[:, lo:hi, :], AF.Relu)

                # ---- primal ----
                for lo, hi in vs:
                    v.tensor_mul(a_(lo, hi), G1[:, lo:hi, :], QH[:, lo:hi, :])
                nc.sync.dma_start(out=At[1:P, 0, :], in_=At[0 : P - 1, NP, :])
                bsplits = vs if asg["b"] == "v" else gs_
                for k, (lo, hi) in enumerate(bsplits):
                    on_v = asg["b"] == "v" or (asg["b"] == "mix"
                                               and k < len(bsplits) // 2)
                    if on_v:
                        v.tensor_mul(b_(lo, hi), G0[:, lo:hi, :], QH[:, lo:hi, :])
                    else:
                        g.tensor_mul(b_(lo, hi), G0[:, lo:hi, :], QH[:, lo:hi, :])

                if pe_mode:
                    # s accumulates on PE in PSUM per 512-col chunk:
                    # dn=True:  s = I@o2 + I@Dn + I@A_up + I@B_left,
                    #           with Dn = -A-B from one Pool STT
                    # dn=False: s = -I@A - I@B + I@o2 + I@A_up + I@B_left
                    assert not asg["dn"], "dn mode needs Pool STT (not on HW)"
                    if asg["s4"]:
                        for lo, hi in vs:
                            v.tensor_sub(C1[:, lo:hi, :], O2[:, lo:hi, :],
                                         a_(lo, hi))
                    sdve = asg["s_dve"]
                    if sdve:                    # trailing planes on DVE
                        lo, hi = NP - sdve, NP
                        v.tensor_sub(t_(lo, hi), O2[:, lo:hi, :], a_(lo, hi))
                        v.tensor_sub(t_(lo, hi), t_(lo, hi), b_(lo, hi))
                        v.tensor_add(t_(lo, hi), t_(lo, hi), a_up(lo, hi))
                        v.tensor_add(t_(lo, hi), t_(lo, hi), b_lf(lo, hi))
                        if not last:
                            act.activation(t_(lo, hi), t_(lo, hi), AF.Tanh,
                                           scale=0.5)
                    n_s = NP - sdve
                    so = asg["s_order"]
                    if isinstance(so, list):
                        order = [j for j in so if j < n_s]
                    else:
                        order = {0: list(range(n_s)),
                                 1: [1, 2, 3, 0] + list(range(4, n_s)),
                                 2: list(reversed(range(n_s)))}[so]
                    for j in order:             # plane: 2-bank ping-pong
                        ps = pq[j % 2]
                        for c in range(0, W, CH):
                            pslab = ps[:, 0, c : c + CH]
                            if asg["dn"]:
                                terms = [(Eye, O2[:, j, c : c + CH]),
                                         (Eye, Dn[:, j, c : c + CH]),
                                         (Eye, At[:, j, c : c + CH]),
                                         (Eye, Bt[:, j, c : c + CH])]
                            elif asg["s4"]:
                                terms = [(Eye, C1[:, j, c : c + CH]),
                                         (NEye, b_(j, j + 1)[:, 0, c : c + CH]),
                                         (Eye, At[:, j, c : c + CH]),
                                         (Eye, Bt[:, j, c : c + CH])]
                            else:
                                terms = [(NEye, a_(j, j + 1)[:, 0, c : c + CH]),
                                         (NEye, b_(j, j + 1)[:, 0, c : c + CH]),
                                         (Eye, O2[:, j, c : c + CH]),
                                         (Eye, At[:, j, c : c + CH]),
                                         (Eye, Bt[:, j, c : c + CH])]
                            for k, (w, src) in enumerate(terms):
                                pe.matmul(pslab, w[:, :], src,
                                          start=(k == 0),
                                          stop=(k == len(terms) - 1))
                        fn = AF.Copy if last else AF.Tanh
                        kw = {} if last else {"scale": 0.5}
                        act.activation(Tt[:, j : j + 1, 0:W], ps[:, :, :],
                                       fn, **kw)
                        if j == 0 and not last:
                            nc.sync.dma_start(
                                out=Tt[0 : P - 1, NP, 0:W], in_=Tt[1:P, 0, 0:W]
                            )
                else:
                    for lo, hi in reversed(vs):
                        v.tensor_sub(t_(lo, hi), O2[:, lo:hi, :], a_(lo, hi))
                    for lo, hi in reversed(vs):
                        v.tensor_sub(t_(lo, hi), t_(lo, hi), b_(lo, hi))
                    for lo, hi in reversed(vs):
                        v.tensor_add(t_(lo, hi), t_(lo, hi), a_up(lo, hi))
                    for lo, hi in reversed(vs):
                        v.tensor_add(t_(lo, hi), t_(lo, hi), b_lf(lo, hi))
                    if not last:
                        for lo, hi in ss:
                            act.activation(t_(lo, hi), t_(lo, hi), AF.Tanh,
                                           scale=0.5)
                            if lo == 0:
                                nc.sync.dma_start(
                                    out=Tt[0 : P - 1, NP, 0:W], in_=Tt[1:P, 0, 0:W]
                                )

            nc.sync.dma_start(out=out_v, in_=Tt[:, 0:NP, 0:W])

    nc.compile()
    return nc


def kernel(o, vector_field, nabla_w, div_w):
    global LAST_RESULTS
    if "nc" not in _CACHE:
        _CACHE["nc"] = _build()
    nc = _CACHE["nc"]

    o2 = np.ascontiguousarray(
        (2.0 * np.asarray(o, dtype=np.float32)[:, 0]).astype(np.float16)
    )
    vf = np.asarray(vector_field, dtype=np.float32)
    s = np.float32(1.0 / np.sqrt(2.0))
    g0 = np.ascontiguousarray((vf[:, :, 0] * s).astype(np.float16))
    g1 = np.ascontiguousarray((vf[:, :, 1] * s).astype(np.float16))
    eye = np.eye(P, dtype=np.float16)

    in_maps = [{"o2": o2[b], "g0": g0, "g1": g1, "eye": eye} for b in range(B)]
    res = bass_utils.run_bass_kernel_spmd(nc, in_maps, core_ids=list(range(B)))
    LAST_RESULTS = res
    return np.stack([r["out"] for r in res.results]).astype(np.float32)


# revision 9
# speedup vs baseline: 1.1610x; 1.0271x over previous
"""Trainium2 Bass kernel v2 — fp16 + PE-accumulated primal, for the CP loop.

Math per image (H=W=1024, 10 iters), tanh/rescaled form (see baseline):
    dual:   qh = relu(qh + gs*t - g1*t_down - g0*t_right)     t pads: -1
    primal: s  = o2 - A - B + A_up + B_left,  A = g1*qh, B = g0*qh, pads 0
            t  = tanh(s/2);  output = s after iteration 10.

Measured: 310,054 ns timeline-sim (fp32 baseline: 1,122,789 ns, 3.6x), HW
rel-L2 9.18e-3 vs the 2e-2 gate (the error is relu-boundary chaos amplifying
fp16 rounding -- an ensemble statistic, stable across reruns).

Key techniques vs the fp32 baseline:
  * fp16 state: DVE 2-byte tensor_tensor runs at 2x (2.2us/half-image op);
    numpy bit-level emulation predicted rel-L2 ~1e-2, HW measures 9.2e-3.
  * shifted-PRODUCT reads: A_up/B_left/t_down/t_right are free AP offsets
    into guard planes/cols; only two tiny boundary-row DMAs per iteration.
  * the primal sum runs on the otherwise-idle PE as identity-weight matmuls
    accumulating in PSUM (fp32), in 512-col chunks:
        s_psum = -I@A - I@B + I@o2 + I@A_up + I@B_left
    and the qh sum for the first qh_pe planes likewise:
        qh_psum = I@qh + I@P1 - I@P2 - I@P3
    relu/tanh then read PSUM directly on ScalarE (cheaper access than SBUF).
    This moves most adds off DVE and upgrades the sums to fp32 accumulation.
  * Pool (gpsimd) takes the P2 = g1*t_down product as a plain tensor_tensor
    (the only elementwise form its HW ISA accepts).

Layout: y = 8p + i -> partition p, plane i; x = free col.  Tiles fp16:
    T  [128,9,1025]: t planes 0..7 cols 0..1023, col 1024 = -1 guard,
                     plane 8 = boundary row t[8p+8] (DMA p+1 -> p each iter)
    A  [128,9,1024]: planes 1..8 = g1*qh, plane 0 = boundary A[8p-1]
    B  [128,8,1025]: cols 1..1024 = g0*qh, col 0 = 0 guard
"""

import numpy as np

import concourse.bacc as bacc
import concourse.mybir as mybir
from concourse.tile import TileContext
from concourse import bass_utils

F16 = mybir.dt.float16
F32 = mybir.dt.float32
AF = mybir.ActivationFunctionType
Alu = mybir.AluOpType

B, H, W = 8, 1024, 1024
P = 128
NP = H // P      # 8 planes per partition
WG = W + 1
MAXITER = 10
CH = 512         # matmul moving-operand chunk

_CACHE = {}
LAST_RESULTS = None

DEFAULT_ASSIGN = {
    "s_mode": "pe",   # 'pe' = PSUM-accumulated primal, 'v' = DVE adds
    "b": "mix",       # engine for B = g0 * qh ('v', 'g', or 'mix': half each)
    "dn": False,      # True: fold -A-B into Dn on Pool (single +I stationary);
                      # False: 5-term MM with +I/-I stationary swaps
    "qh_pe": 6,       # number of planes (0..8) whose qh-sum runs on PE
    "qh3": False,     # True: R = P1+P2n on DVE -> 3-term qh MM
    "s4": False,      # True: C1 = o2-A on DVE -> 4-term s MM
    "s_dve": 1,       # number of trailing planes whose s-sum runs on DVE
    "p2": "mix",      # engine for P2 = g1*t_down ('g', 'v', 'mix')
    "a": "v",         # engine for A = g1*qh ('v' or 'mix': half each)
    # s-plane order: run plane 0 third so its A_up boundary-row DMA (which
    # needs the last A quarter) is complete by the time its matmuls issue
    "s_order": [1, 2, 0, 3, 4, 5, 6, 7],
    "v_split": 4,
    "g_split": 4,
    "s_split": 4,
}


def _build(assign=None):
    asg = dict(DEFAULT_ASSIGN)
    if assign:
        asg.update(assign)

    nc = bacc.Bacc("TRN2", target_bir_lowering=False, debug=False)

    o2_d = nc.dram_tensor("o2", [H, W], F16, kind="ExternalInput").ap()
    g0_d = nc.dram_tensor("g0", [H, W], F16, kind="ExternalInput").ap()
    g1_d = nc.dram_tensor("g1", [H, W], F16, kind="ExternalInput").ap()
    eye_d = nc.dram_tensor("eye", [P, P], F16, kind="ExternalInput").ap()
    out_d = nc.dram_tensor("out", [H, W], F16, kind="ExternalOutput").ap()

    o2_v = o2_d.rearrange("(p i) x -> p i x", i=NP)
    g0_v = g0_d.rearrange("(p i) x -> p i x", i=NP)
    g1_v = g1_d.rearrange("(p i) x -> p i x", i=NP)
    out_v = out_d.rearrange("(p i) x -> p i x", i=NP)

    v = nc.vector
    g = nc.gpsimd
    act = nc.scalar
    pe = nc.tensor

    with TileContext(nc) as tc:
        with tc.tile_pool(name="main", bufs=1) as pool, \
             tc.psum_pool(name="ps", bufs=1) as ppool:
            Tt = pool.tile([P, NP + 1, WG], F16)
            QH = pool.tile([P, NP, W], F16)
            O2 = pool.tile([P, NP, W], F16)
            GS = pool.tile([P, NP, W], F16)
            G1 = pool.tile([P, NP, W], F16)
            G0 = pool.tile([P, NP, W], F16)
            At = pool.tile([P, NP + 1, W], F16)
            Bt = pool.tile([P, NP, WG], F16)
            P1t = pool.tile([P, NP, W], F16)
            P2t = pool.tile([P, NP, W], F16)
            P3t = pool.tile([P, NP, W], F16)
            pe_mode = asg["s_mode"] == "pe"
            if pe_mode:
                Eye = pool.tile([P, P], F16)
                pq = [ppool.tile([P, 1, W], F32, name=f"pq{i}") for i in range(2)]
                if asg["qh_pe"]:
                    qpq = [ppool.tile([P, 1, W], F32, name=f"qpq{i}")
                           for i in range(2)]
                NEye = pool.tile([P, P], F16)
                if asg["s4"]:
                    C1 = pool.tile([P, NP, W], F16)

            def spl(n):
                step = NP // n
                return [(i * step, (i + 1) * step) for i in range(n)]

            vs = spl(asg["v_split"])
            gs_ = spl(asg["g_split"])
            ss = spl(asg["s_split"])
            halves = spl(2)

            def t_(lo, hi):
                return Tt[:, lo:hi, 0:W]

            def t_dn(lo, hi):   # t[y+1]; plane 8 = boundary
                return Tt[:, lo + 1 : hi + 1, 0:W]

            def t_rt(lo, hi):   # t[y, x+1]; col W = -1 guard
                return Tt[:, lo:hi, 1 : W + 1]

            def a_(lo, hi):     # A[y] (data planes 1..8)
                return At[:, lo + 1 : hi + 1, :]

            def a_up(lo, hi):   # A[y-1] (plane 0 = boundary)
                return At[:, lo:hi, :]

            def b_(lo, hi):     # B[y] (data cols 1..W)
                return Bt[:, lo:hi, 1 : W + 1]

            def b_lf(lo, hi):   # B[y, x-1] (col 0 = 0 guard)
                return Bt[:, lo:hi, 0:W]

            # --- setup ---
            # Guard-only memsets: t data planes are written by the first tanh,
            # qh by iteration 0 (which skips the qh term), A/B data each iter.
            v.memset(Tt[:, 0:NP, W : W + 1], -1.0)   # t_right -1 guard col
            v.memset(Tt[:, NP, :], -1.0)             # t_down boundary plane
            v.memset(At[:, 0, :], 0.0)               # A_up zero pad (part. 0)
            v.memset(Bt[:, :, 0:1], 0.0)             # B_left zero guard col
            # Half-split loads so the first tanh/products start early.
            for lo, hi in halves:
                nc.sync.dma_start(out=O2[:, lo:hi, :], in_=o2_v[:, lo:hi, :])
                nc.sync.dma_start(out=G0[:, lo:hi, :], in_=g0_v[:, lo:hi, :])
                nc.sync.dma_start(out=G1[:, lo:hi, :], in_=g1_v[:, lo:hi, :])
                if pe_mode and lo == 0:
                    nc.sync.dma_start(out=Eye[:, :], in_=eye_d)
            if pe_mode:
                v.tensor_scalar_mul(NEye[:, :], Eye[:, :], -1.0)
            for lo, hi in halves:
                v.tensor_add(GS[:, lo:hi, :], G0[:, lo:hi, :], G1[:, lo:hi, :])
            for lo, hi in halves:
                act.activation(t_(lo, hi), O2[:, lo:hi, :], AF.Tanh, scale=0.5)
            nc.sync.dma_start(out=Tt[0 : P - 1, NP, 0:W], in_=Tt[1:P, 0, 0:W])

            for it in range(MAXITER):
                last = it == MAXITER - 1

                # ---- dual: qh' = relu(qh + P1 - P2 - P3) ----
                # Products are positive (Pool only supports plain
                # tensor_tensor on HW); minus signs come from DVE subtracts or
                # the -I stationary on PE planes.
                qpe = asg["qh_pe"] if pe_mode else 0
                p2splits = gs_ if asg["p2"] == "g" else vs
                for k, (lo, hi) in enumerate(p2splits):
                    on_g = asg["p2"] == "g" or (asg["p2"] == "mix"
                                                and k >= len(p2splits) // 2)
                    if on_g:
                        g.tensor_mul(P2t[:, lo:hi, :], t_dn(lo, hi),
                                     G1[:, lo:hi, :])
                    else:
                        v.tensor_mul(P2t[:, lo:hi, :], t_dn(lo, hi),
                                     G1[:, lo:hi, :])
                for lo, hi in vs:
                    v.tensor_mul(P1t[:, lo:hi, :], GS[:, lo:hi, :], t_(lo, hi))
                for lo, hi in vs:
                    v.tensor_mul(P3t[:, lo:hi, :], G0[:, lo:hi, :], t_rt(lo, hi))
                for j in range(qpe):            # PE planes: qh-sum in PSUM
                    qp = qpq[j % 2]
                    for c in range(0, W, CH):
                        pslab = qp[:, 0, c : c + CH]
                        terms = [] if it == 0 else [(Eye, QH[:, j, c : c + CH])]
                        terms += [(Eye, P1t[:, j, c : c + CH]),
                                  (NEye, P2t[:, j, c : c + CH]),
                                  (NEye, P3t[:, j, c : c + CH])]
                        for k, (w, src) in enumerate(terms):
                            pe.matmul(pslab, w[:, :], src, start=(k == 0),
                                      stop=(k == len(terms) - 1))
                    act.activation(QH[:, j : j + 1, :], qp[:, :, :], AF.Relu)
                if qpe < NP:                    # DVE planes: in-place adds
                    dvs = [(max(lo, qpe), hi) for lo, hi in vs if hi > qpe]
                    if it == 0:
                        for lo, hi in dvs:
                            v.tensor_sub(QH[:, lo:hi, :], P1t[:, lo:hi, :],
                                         P3t[:, lo:hi, :])
                    else:
                        for lo, hi in dvs:
                            v.tensor_add(QH[:, lo:hi, :], QH[:, lo:hi, :],
                                         P1t[:, lo:hi, :])
                        for lo, hi in dvs:
                            v.tensor_sub(QH[:, lo:hi, :], QH[:, lo:hi, :],
                                         P3t[:, lo:hi, :])
                    for lo, hi in dvs:
                        v.tensor_sub(QH[:, lo:hi, :], QH[:, lo:hi, :],
                                     P2t[:, lo:hi, :])
                    for lo, hi in [(max(lo, qpe), hi) for lo, hi in ss
                                   if hi > qpe]:
                        act.activation(QH[:, lo:hi, :], QH[:, lo:hi, :], AF.Relu)

                # ---- primal ----
                asplits = vs if asg["a"] == "v" else gs_
                for k, (lo, hi) in enumerate(asplits):
                    if asg["a"] == "v" or k < len(asplits) // 2:
                        v.tensor_mul(a_(lo, hi), G1[:, lo:hi, :],
                                     QH[:, lo:hi, :])
                    else:
                        g.tensor_mul(a_(lo, hi), G1[:, lo:hi, :],
                                     QH[:, lo:hi, :])
                nc.sync.dma_start(out=At[1:P, 0, :], in_=At[0 : P - 1, NP, :])
                bsplits = vs if asg["b"] == "v" else gs_
                for k, (lo, hi) in enumerate(bsplits):
                    on_v = asg["b"] == "v" or (asg["b"] == "mix"
                                               and k < len(bsplits) // 2)
                    if on_v:
                        v.tensor_mul(b_(lo, hi), G0[:, lo:hi, :], QH[:, lo:hi, :])
                    else:
                        g.tensor_mul(b_(lo, hi), G0[:, lo:hi, :], QH[:, lo:hi, :])

                if pe_mode:
                    # s accumulates on PE in PSUM per 512-col chunk:
                    # dn=True:  s = I@o2 + I@Dn + I@A_up + I@B_left,
                    #           with Dn = -A-B from one Pool STT
                    # dn=False: s = -I@A - I@B + I@o2 + I@A_up + I@B_left
                    assert not asg["dn"], "dn mode needs Pool STT (not on HW)"
                    if asg["s4"]:
                        for lo, hi in vs:
                            v.tensor_sub(C1[:, lo:hi, :], O2[:, lo:hi, :],
                                         a_(lo, hi))
                    sdve = asg["s_dve"]
                    if sdve:                    # trailing planes on DVE
                        lo, hi = NP - sdve, NP
                        v.tensor_sub(t_(lo, hi), O2[:, lo:hi, :], a_(lo, hi))
                        v.tensor_sub(t_(lo, hi), t_(lo, hi), b_(lo, hi))
                        v.tensor_add(t_(lo, hi), t_(lo, hi), a_up(lo, hi))
                        v.tensor_add(t_(lo, hi), t_(lo, hi), b_lf(lo, hi))
                        if not last:
                            act.activation(t_(lo, hi), t_(lo, hi), AF.Tanh,
                                           scale=0.5)
                        else:
                            nc.sync.dma_start(out=out_v[:, lo:hi, :],
                                              in_=t_(lo, hi))
                    n_s = NP - sdve
                    so = asg["s_order"]
                    if isinstance(so, list):
                        order = [j for j in so if j < n_s]
                    else:
                        order = {0: list(range(n_s)),
                                 1: [1, 2, 3, 0] + list(range(4, n_s)),
                                 2: list(reversed(range(n_s)))}[so]
                    for j in order:             # plane: 2-bank ping-pong
                        ps = pq[j % 2]
                        for c in range(0, W, CH):
                            pslab = ps[:, 0, c : c + CH]
                            if asg["dn"]:
                                terms = [(Eye, O2[:, j, c : c + CH]),
                                         (Eye, Dn[:, j, c : c + CH]),
                                         (Eye, At[:, j, c : c + CH]),
                                         (Eye, Bt[:, j, c : c + CH])]
                            elif asg["s4"]:
                                terms = [(Eye, C1[:, j, c : c + CH]),
                                         (NEye, b_(j, j + 1)[:, 0, c : c + CH]),
                                         (Eye, At[:, j, c : c + CH]),
                                         (Eye, Bt[:, j, c : c + CH])]
                            else:
                                terms = [(NEye, a_(j, j + 1)[:, 0, c : c + CH]),
                                         (NEye, b_(j, j + 1)[:, 0, c : c + CH]),
                                         (Eye, O2[:, j, c : c + CH]),
                                         (Eye, At[:, j, c : c + CH]),
                                         (Eye, Bt[:, j, c : c + CH])]
                            for k, (w, src) in enumerate(terms):
                                pe.matmul(pslab, w[:, :], src,
                                          start=(k == 0),
                                          stop=(k == len(terms) - 1))
                        fn = AF.Copy if last else AF.Tanh
                        kw = {} if last else {"scale": 0.5}
                        act.activation(Tt[:, j : j + 1, 0:W], ps[:, :, :],
                                       fn, **kw)
                        if last:
                            nc.sync.dma_start(out=out_v[:, j, :],
                                              in_=Tt[:, j, 0:W])
                        elif j == 0:
                            nc.sync.dma_start(
                                out=Tt[0 : P - 1, NP, 0:W], in_=Tt[1:P, 0, 0:W]
                            )
                else:
                    for lo, hi in reversed(vs):
                        v.tensor_sub(t_(lo, hi), O2[:, lo:hi, :], a_(lo, hi))
                    for lo, hi in reversed(vs):
                        v.tensor_sub(t_(lo, hi), t_(lo, hi), b_(lo, hi))
                    for lo, hi in reversed(vs):
                        v.tensor_add(t_(lo, hi), t_(lo, hi), a_up(lo, hi))
                    for lo, hi in reversed(vs):
                        v.tensor_add(t_(lo, hi), t_(lo, hi), b_lf(lo, hi))
                    if not last:
                        for lo, hi in ss:
                            act.activation(t_(lo, hi), t_(lo, hi), AF.Tanh,
                                           scale=0.5)
                            if lo == 0:
                                nc.sync.dma_start(
                                    out=Tt[0 : P - 1, NP, 0:W], in_=Tt[1:P, 0, 0:W]
                                )

            if not pe_mode:
                nc.sync.dma_start(out=out_v, in_=Tt[:, 0:NP, 0:W])

    nc.compile()
    return nc


def kernel(o, vector_field, nabla_w, div_w):
    global LAST_RESULTS
    if "nc" not in _CACHE:
        _CACHE["nc"] = _build()
    nc = _CACHE["nc"]

    o2 = np.ascontiguousarray(
        (2.0 * np.asarray(o, dtype=np.float32)[:, 0]).astype(np.float16)
    )
    vf = np.asarray(vector_field, dtype=np.float32)
    s = np.float32(1.0 / np.sqrt(2.0))
    g0 = np.ascontiguousarray((vf[:, :, 0] * s).astype(np.float16))
    g1 = np.ascontiguousarray((vf[:, :, 1] * s).astype(np.float16))
    eye = np.eye(P, dtype=np.float16)

    in_maps = [{"o2": o2[b], "g0": g0, "g1": g1, "eye": eye} for b in range(B)]
    res = bass_utils.run_bass_kernel_spmd(nc, in_maps, core_ids=list(range(B)))
    LAST_RESULTS = res
    return np.stack([r["out"] for r in res.results]).astype(np.float32)


# revision 12
# speedup vs baseline: 1.2041x; 1.0371x over previous
"""Trainium2 Bass kernel v2 — fp16 + PE-accumulated primal, for the CP loop.

Math per image (H=W=1024, 10 iters), tanh/rescaled form (see baseline):
    dual:   qh = relu(qh + gs*t - g1*t_down - g0*t_right)     t pads: -1
    primal: s  = o2 - A - B + A_up + B_left,  A = g1*qh, B = g0*qh, pads 0
            t  = tanh(s/2);  output = s after iteration 10.

Measured: 301,880 ns timeline-sim (fp32 baseline: 1,122,789 ns, 3.72x), HW
rel-L2 9.18e-3 vs the 2e-2 gate (the error is relu-boundary chaos amplifying
fp16 rounding -- an ensemble statistic, stable across reruns).  In steady
state the PE is the iteration clock (96% busy within the loop) with DVE at
92%; startup/drain are trimmed via guard-only memsets, half-split input
loads, and per-plane output DMAs.

Key techniques vs the fp32 baseline:
  * fp16 state: DVE 2-byte tensor_tensor runs at 2x (2.2us/half-image op);
    numpy bit-level emulation predicted rel-L2 ~1e-2, HW measures 9.2e-3.
  * shifted-PRODUCT reads: A_up/B_left/t_down/t_right are free AP offsets
    into guard planes/cols; only two tiny boundary-row DMAs per iteration.
  * the primal sum runs on the otherwise-idle PE as identity-weight matmuls
    accumulating in PSUM (fp32), in 512-col chunks:
        s_psum = -I@A - I@B + I@o2 + I@A_up + I@B_left
    and the qh sum for the first qh_pe planes likewise:
        qh_psum = I@qh + I@P1 - I@P2 - I@P3
    relu/tanh then read PSUM directly on ScalarE (cheaper access than SBUF).
    This moves most adds off DVE and upgrades the sums to fp32 accumulation.
  * Pool (gpsimd) takes the P2 = g1*t_down product as a plain tensor_tensor
    (the only elementwise form its HW ISA accepts).

Layout: y = 8p + i -> partition p, plane i; x = free col.  Tiles fp16:
    T  [128,9,1025]: t planes 0..7 cols 0..1023, col 1024 = -1 guard,
                     plane 8 = boundary row t[8p+8] (DMA p+1 -> p each iter)
    A  [128,9,1024]: planes 1..8 = g1*qh, plane 0 = boundary A[8p-1]
    B  [128,8,1025]: cols 1..1024 = g0*qh, col 0 = 0 guard
"""

import numpy as np

import concourse.bacc as bacc
import concourse.mybir as mybir
from concourse.tile import TileContext
from concourse import bass_utils

F16 = mybir.dt.float16
F32 = mybir.dt.float32
AF = mybir.ActivationFunctionType
Alu = mybir.AluOpType

B, H, W = 8, 1024, 1024
P = 128
NP = H // P      # 8 planes per partition
WG = W + 1
MAXITER = 10
CH = 512         # matmul moving-operand chunk

_CACHE = {}
LAST_RESULTS = None

DEFAULT_ASSIGN = {
    "s_mode": "pe",   # 'pe' = PSUM-accumulated primal, 'v' = DVE adds
    "b": "mix",       # engine for B = g0 * qh ('v', 'g', or 'mix': half each)
    "dn": False,      # True: fold -A-B into Dn on Pool (single +I stationary);
                      # False: 5-term MM with +I/-I stationary swaps
    "qh_pe": 6,       # number of planes (0..8) whose qh-sum runs on PE
    "qh3": False,     # True: R = P1+P2n on DVE -> 3-term qh MM
    "s4": False,      # True: C1 = o2-A on DVE -> 4-term s MM
    "s_dve": 1,       # number of trailing planes whose s-sum runs on DVE
    "p2": "mix",      # engine for P2 = g1*t_down ('g', 'v', 'mix')
    "a": "v",         # engine for A = g1*qh ('v' or 'mix': half each)
    "qh_p2_pool": False,  # True: DVE-planes' qh -= P2 via Pool tensor_tensor
    "b_vq": None,     # override: number of leading B quarters on DVE
    # s-plane order: run plane 0 third so its A_up boundary-row DMA (which
    # needs the last A quarter) is complete by the time its matmuls issue
    "s_order": [1, 2, 0, 3, 4, 5, 6, 7],
    "v_split": 4,
    "g_split": 4,
    "s_split": 4,
}


def _build(assign=None):
    asg = dict(DEFAULT_ASSIGN)
    if assign:
        asg.update(assign)

    nc = bacc.Bacc("TRN2", target_bir_lowering=False, debug=False)

    o2_d = nc.dram_tensor("o2", [H, W], F16, kind="ExternalInput").ap()
    g0_d = nc.dram_tensor("g0", [H, W], F16, kind="ExternalInput").ap()
    g1_d = nc.dram_tensor("g1", [H, W], F16, kind="ExternalInput").ap()
    eye_d = nc.dram_tensor("eye", [P, P], F16, kind="ExternalInput").ap()
    out_d = nc.dram_tensor("out", [H, W], F16, kind="ExternalOutput").ap()

    o2_v = o2_d.rearrange("(p i) x -> p i x", i=NP)
    g0_v = g0_d.rearrange("(p i) x -> p i x", i=NP)
    g1_v = g1_d.rearrange("(p i) x -> p i x", i=NP)
    out_v = out_d.rearrange("(p i) x -> p i x", i=NP)

    v = nc.vector
    g = nc.gpsimd
    act = nc.scalar
    pe = nc.tensor

    with TileContext(nc) as tc:
        with tc.tile_pool(name="main", bufs=1) as pool, \
             tc.psum_pool(name="ps", bufs=1) as ppool:
            Tt = pool.tile([P, NP + 1, WG], F16)
            QH = pool.tile([P, NP, W], F16)
            O2 = pool.tile([P, NP, W], F16)
            GS = pool.tile([P, NP, W], F16)
            G1 = pool.tile([P, NP, W], F16)
            G0 = pool.tile([P, NP, W], F16)
            At = pool.tile([P, NP + 1, W], F16)
            Bt = pool.tile([P, NP, WG], F16)
            P1t = pool.tile([P, NP, W], F16)
            P2t = pool.tile([P, NP, W], F16)
            P3t = pool.tile([P, NP, W], F16)
            pe_mode = asg["s_mode"] == "pe"
            if pe_mode:
                Eye = pool.tile([P, P], F16)
                pq = [ppool.tile([P, 1, W], F32, name=f"pq{i}") for i in range(2)]
                if asg["qh_pe"]:
                    qpq = [ppool.tile([P, 1, W], F32, name=f"qpq{i}")
                           for i in range(2)]
                NEye = pool.tile([P, P], F16)
                if asg["s4"]:
                    C1 = pool.tile([P, NP, W], F16)

            def spl(n):
                step = NP // n
                return [(i * step, (i + 1) * step) for i in range(n)]

            vs = spl(asg["v_split"])
            gs_ = spl(asg["g_split"])
            ss = spl(asg["s_split"])
            halves = spl(2)

            def t_(lo, hi):
                return Tt[:, lo:hi, 0:W]

            def t_dn(lo, hi):   # t[y+1]; plane 8 = boundary
                return Tt[:, lo + 1 : hi + 1, 0:W]

            def t_rt(lo, hi):   # t[y, x+1]; col W = -1 guard
                return Tt[:, lo:hi, 1 : W + 1]

            def a_(lo, hi):     # A[y] (data planes 1..8)
                return At[:, lo + 1 : hi + 1, :]

            def a_up(lo, hi):   # A[y-1] (plane 0 = boundary)
                return At[:, lo:hi, :]

            def b_(lo, hi):     # B[y] (data cols 1..W)
                return Bt[:, lo:hi, 1 : W + 1]

            def b_lf(lo, hi):   # B[y, x-1] (col 0 = 0 guard)
                return Bt[:, lo:hi, 0:W]

            # --- setup ---
            # Full state memsets run on the otherwise-idle Pool engine so they
            # hide entirely under the input loads (defensive: uninitialized
            # SBUF showed up as intermittent corruption on hardware).
            g.memset(Tt[:, :, :], -1.0)              # t guards/boundary = -1
            g.memset(QH[:, :, :], 0.0)
            v.memset(At[:, 0, :], 0.0)               # A_up zero pad (part. 0)
            v.memset(Bt[:, :, 0:1], 0.0)             # B_left zero guard col
            # Half-split loads so the first tanh/products start early.
            for lo, hi in halves:
                nc.sync.dma_start(out=O2[:, lo:hi, :], in_=o2_v[:, lo:hi, :])
                nc.sync.dma_start(out=G0[:, lo:hi, :], in_=g0_v[:, lo:hi, :])
                nc.sync.dma_start(out=G1[:, lo:hi, :], in_=g1_v[:, lo:hi, :])
                if pe_mode and lo == 0:
                    nc.sync.dma_start(out=Eye[:, :], in_=eye_d)
            if pe_mode:
                v.tensor_scalar_mul(NEye[:, :], Eye[:, :], -1.0)
            for lo, hi in halves:
                v.tensor_add(GS[:, lo:hi, :], G0[:, lo:hi, :], G1[:, lo:hi, :])
            for lo, hi in halves:
                act.activation(t_(lo, hi), O2[:, lo:hi, :], AF.Tanh, scale=0.5)
            nc.sync.dma_start(out=Tt[0 : P - 1, NP, 0:W], in_=Tt[1:P, 0, 0:W])

            for it in range(MAXITER):
                last = it == MAXITER - 1

                # ---- dual: qh' = relu(qh + P1 - P2 - P3) ----
                # Products are positive (Pool only supports plain
                # tensor_tensor on HW); minus signs come from DVE subtracts or
                # the -I stationary on PE planes.
                qpe = asg["qh_pe"] if pe_mode else 0
                p2splits = gs_ if asg["p2"] == "g" else vs
                for k, (lo, hi) in enumerate(p2splits):
                    on_g = asg["p2"] == "g" or (asg["p2"] == "mix"
                                                and k >= len(p2splits) // 2)
                    if on_g:
                        g.tensor_mul(P2t[:, lo:hi, :], t_dn(lo, hi),
                                     G1[:, lo:hi, :])
                    else:
                        v.tensor_mul(P2t[:, lo:hi, :], t_dn(lo, hi),
                                     G1[:, lo:hi, :])
                for lo, hi in vs:
                    v.tensor_mul(P1t[:, lo:hi, :], GS[:, lo:hi, :], t_(lo, hi))
                for lo, hi in vs:
                    v.tensor_mul(P3t[:, lo:hi, :], G0[:, lo:hi, :], t_rt(lo, hi))
                for j in range(qpe):            # PE planes: qh-sum in PSUM
                    qp = qpq[j % 2]
                    for c in range(0, W, CH):
                        pslab = qp[:, 0, c : c + CH]
                        terms = [] if it == 0 else [(Eye, QH[:, j, c : c + CH])]
                        terms += [(Eye, P1t[:, j, c : c + CH]),
                                  (NEye, P2t[:, j, c : c + CH]),
                                  (NEye, P3t[:, j, c : c + CH])]
                        for k, (w, src) in enumerate(terms):
                            pe.matmul(pslab, w[:, :], src, start=(k == 0),
                                      stop=(k == len(terms) - 1))
                    act.activation(QH[:, j : j + 1, :], qp[:, :, :], AF.Relu)
                if qpe < NP:                    # DVE planes: in-place adds
                    dvs = [(max(lo, qpe), hi) for lo, hi in vs if hi > qpe]
                    if it == 0:
                        for lo, hi in dvs:
                            v.tensor_sub(QH[:, lo:hi, :], P1t[:, lo:hi, :],
                                         P3t[:, lo:hi, :])
                    else:
                        for lo, hi in dvs:
                            v.tensor_add(QH[:, lo:hi, :], QH[:, lo:hi, :],
                                         P1t[:, lo:hi, :])
                        for lo, hi in dvs:
                            v.tensor_sub(QH[:, lo:hi, :], QH[:, lo:hi, :],
                                         P3t[:, lo:hi, :])
                    for lo, hi in dvs:
                        if asg["qh_p2_pool"]:
                            g.tensor_sub(QH[:, lo:hi, :], QH[:, lo:hi, :],
                                         P2t[:, lo:hi, :])
                        else:
                            v.tensor_sub(QH[:, lo:hi, :], QH[:, lo:hi, :],
                                         P2t[:, lo:hi, :])
                    for lo, hi in [(max(lo, qpe), hi) for lo, hi in ss
                                   if hi > qpe]:
                        act.activation(QH[:, lo:hi, :], QH[:, lo:hi, :], AF.Relu)

                # ---- primal ----
                asplits = vs if asg["a"] == "v" else gs_
                for k, (lo, hi) in enumerate(asplits):
                    if asg["a"] == "v" or k < len(asplits) // 2:
                        v.tensor_mul(a_(lo, hi), G1[:, lo:hi, :],
                                     QH[:, lo:hi, :])
                    else:
                        g.tensor_mul(a_(lo, hi), G1[:, lo:hi, :],
                                     QH[:, lo:hi, :])
                nc.sync.dma_start(out=At[1:P, 0, :], in_=At[0 : P - 1, NP, :])
                bsplits = vs if asg["b"] == "v" else gs_
                for k, (lo, hi) in enumerate(bsplits):
                    if asg["b_vq"] is not None:
                        on_v = k < asg["b_vq"]
                    else:
                        on_v = asg["b"] == "v" or (asg["b"] == "mix"
                                                   and k < len(bsplits) // 2)
                    if on_v:
                        v.tensor_mul(b_(lo, hi), G0[:, lo:hi, :], QH[:, lo:hi, :])
                    else:
                        g.tensor_mul(b_(lo, hi), G0[:, lo:hi, :], QH[:, lo:hi, :])

                if pe_mode:
                    # s accumulates on PE in PSUM per 512-col chunk:
                    # dn=True:  s = I@o2 + I@Dn + I@A_up + I@B_left,
                    #           with Dn = -A-B from one Pool STT
                    # dn=False: s = -I@A - I@B + I@o2 + I@A_up + I@B_left
                    assert not asg["dn"], "dn mode needs Pool STT (not on HW)"
                    if asg["s4"]:
                        for lo, hi in vs:
                            v.tensor_sub(C1[:, lo:hi, :], O2[:, lo:hi, :],
                                         a_(lo, hi))
                    sdve = asg["s_dve"]
                    if sdve:                    # trailing planes on DVE
                        lo, hi = NP - sdve, NP
                        v.tensor_sub(t_(lo, hi), O2[:, lo:hi, :], a_(lo, hi))
                        v.tensor_sub(t_(lo, hi), t_(lo, hi), b_(lo, hi))
                        v.tensor_add(t_(lo, hi), t_(lo, hi), a_up(lo, hi))
                        v.tensor_add(t_(lo, hi), t_(lo, hi), b_lf(lo, hi))
                        if not last:
                            act.activation(t_(lo, hi), t_(lo, hi), AF.Tanh,
                                           scale=0.5)
                        else:
                            nc.sync.dma_start(out=out_v[:, lo:hi, :],
                                              in_=t_(lo, hi))
                    n_s = NP - sdve
                    so = asg["s_order"]
                    if isinstance(so, list):
                        order = [j for j in so if j < n_s]
                    else:
                        order = {0: list(range(n_s)),
                                 1: [1, 2, 3, 0] + list(range(4, n_s)),
                                 2: list(reversed(range(n_s)))}[so]
                    for j in order:             # plane: 2-bank ping-pong
                        ps = pq[j % 2]
                        for c in range(0, W, CH):
                            pslab = ps[:, 0, c : c + CH]
                            if asg["dn"]:
                                terms = [(Eye, O2[:, j, c : c + CH]),
                                         (Eye, Dn[:, j, c : c + CH]),
                                         (Eye, At[:, j, c : c + CH]),
                                         (Eye, Bt[:, j, c : c + CH])]
                            elif asg["s4"]:
                                terms = [(Eye, C1[:, j, c : c + CH]),
                                         (NEye, b_(j, j + 1)[:, 0, c : c + CH]),
                                         (Eye, At[:, j, c : c + CH]),
                                         (Eye, Bt[:, j, c : c + CH])]
                            else:
                                terms = [(NEye, a_(j, j + 1)[:, 0, c : c + CH]),
                                         (NEye, b_(j, j + 1)[:, 0, c : c + CH]),
                                         (Eye, O2[:, j, c : c + CH]),
                                         (Eye, At[:, j, c : c + CH]),
                                         (Eye, Bt[:, j, c : c + CH])]
                            for k, (w, src) in enumerate(terms):
                                pe.matmul(pslab, w[:, :], src,
                                          start=(k == 0),
                                          stop=(k == len(terms) - 1))
                        fn = AF.Copy if last else AF.Tanh
                        kw = {} if last else {"scale": 0.5}
                        act.activation(Tt[:, j : j + 1, 0:W], ps[:, :, :],
                                       fn, **kw)
                        if last:
                            nc.sync.dma_start(out=out_v[:, j, :],
                                              in_=Tt[:, j, 0:W])
                        elif j == 0:
                            nc.sync.dma_start(
                                out=Tt[0 : P - 1, NP, 0:W], in_=Tt[1:P, 0, 0:W]
                            )
                else:
                    for lo, hi in reversed(vs):
                        v.tensor_sub(t_(lo, hi), O2[:, lo:hi, :], a_(lo, hi))
                    for lo, hi in reversed(vs):
                        v.tensor_sub(t_(lo, hi), t_(lo, hi), b_(lo, hi))
                    for lo, hi in reversed(vs):
                        v.tensor_add(t_(lo, hi), t_(lo, hi), a_up(lo, hi))
                    for lo, hi in reversed(vs):
                        v.tensor_add(t_(lo, hi), t_(lo, hi), b_lf(lo, hi))
                    if not last:
                        for lo, hi in ss:
                            act.activation(t_(lo, hi), t_(lo, hi), AF.Tanh,
                                           scale=0.5)
                            if lo == 0:
                                nc.sync.dma_start(
                                    out=Tt[0 : P - 1, NP, 0:W], in_=Tt[1:P, 0, 0:W]
                                )

            if not pe_mode:
                nc.sync.dma_start(out=out_v, in_=Tt[:, 0:NP, 0:W])

    nc.compile()
    return nc


def kernel(o, vector_field, nabla_w, div_w):
    global LAST_RESULTS
    if "nc" not in _CACHE:
        _CACHE["nc"] = _build()
    nc = _CACHE["nc"]

    o2 = np.ascontiguousarray(
        (2.0 * np.asarray(o, dtype=np.float32)[:, 0]).astype(np.float16)
    )
    vf = np.asarray(vector_field, dtype=np.float32)
    s = np.float32(1.0 / np.sqrt(2.0))
    g0 = np.ascontiguousarray((vf[:, :, 0] * s).astype(np.float16))
    g1 = np.ascontiguousarray((vf[:, :, 1] * s).astype(np.float16))
    eye = np.eye(P, dtype=np.float16)

    in_maps = [{"o2": o2[b], "g0": g0, "g1": g1, "eye": eye} for b in range(B)]
    res = bass_utils.run_bass_kernel_spmd(nc, in_maps, core_ids=list(range(B)))
    LAST_RESULTS = res
    return np.stack([r["out"] for r in res.results]).astype(np.float32)
